# revision 4
# baseline (speedup 1.0000x reference)
"""AgentWiseFusion Trainium2 kernel.

Strategy (8 NeuronCores, spatial sharding):
  Every core takes a 1/8 slice of the H dimension (8 of 64 rows) of ALL
  agents, so the per-core program is identical (true SPMD) and the ragged
  group structure is baked into the instruction stream.

  The reference computes, per ragged group (sizes from record_len):
    qkv projection -> tiny [b,b] agent attention -> only ego row (row 0)
    of att is used:  out_g = (att[0] @ V).reshape @ w_out + b_out
  and also returns the LAST group's full [b,b] att matrix.

  Algebra used here:
    logits l_j = <M^T x_0, x_j> * scale   with M = w_q @ w_k^T  ([C,C])
    out_g^T    = sum_j att_j * (Wvo^T x_j) + b_out
               with Wvo = w_v @ w_out     ([C,C])
  so the huge Q/K/V tensors are never materialized; x stays channel-major
  exactly as stored in HBM.

  Launch A (device): U = M^T x_lead (fp32 matmuls, PSUM-accumulated) and
    fused dot products <U, x_j> via DVE scalar_tensor_tensor accum.
    Outputs per-core partial logits (all groups + extra rows for the last
    group's full att matrix).
  Host: sums the 8 partial-logit vectors, softmax in float64, producing
    att weights (and the returned att matrix of the last group).
  Launch B (device): out_g^T = sum_j (att_j*Wvo)^T x_j + b_out, with
    att_j*Wvo scaled on-device (DVE) into float32r stationary tiles;
    float32r matmuls (full-rate on PE, ~2^-13 relative error, plenty for
    the value path) accumulate the whole group in PSUM, so no weighted
    elementwise sum is ever needed.
"""
import numpy as np

import concourse.bacc as bacc
import concourse.mybir as mybir
from concourse import tile
from concourse.bass_utils import run_bass_kernel_spmd

FP32 = mybir.dt.float32
FP32R = mybir.dt.float32r
AF = mybir.ActivationFunctionType
OP = mybir.AluOpType

N_CORES = 8
SCALE = 0.17677669529663687  # 1/sqrt(32)


def _group_layout(record_len):
    rl = [int(v) for v in np.asarray(record_len)]
    starts = np.concatenate([[0], np.cumsum(rl)]).astype(int)
    return rl, starts


def build_program_a(rl, C, SP):
    """Launch A: partial logits. SP = spatial elements per core slice."""
    n_agents = sum(rl)
    n_groups = len(rl)
    b_last = rl[-1]
    # dot columns: one per agent (row-0 dots for every group), then
    # (b_last-1)*b_last extra for rows 1.. of the last group's att.
    n_extra = (b_last - 1) * b_last
    n_dots = n_agents + n_extra
    starts = np.concatenate([[0], np.cumsum(rl)]).astype(int)

    CB = C // 128          # channel blocks (2)
    AG = CB * SP           # sbuf columns per agent
    SPC = 512              # psum chunk of spatial
    n_spc = SP // SPC

    nc = bacc.Bacc("TRN2", target_bir_lowering=False, debug=False,
                   num_devices=N_CORES)
    x_in = nc.dram_tensor("xs", [n_agents, C, SP], FP32, kind="ExternalInput")
    m_in = nc.dram_tensor("m", [128, CB * C], FP32, kind="ExternalInput")
    dots_out = nc.dram_tensor("dots", [1, n_dots], FP32, kind="ExternalOutput")

    with tile.TileContext(nc) as tc:
        with (
            tc.tile_pool(name="consts", bufs=1) as consts,
            tc.tile_pool(name="xg", bufs=3) as xgp,
            tc.tile_pool(name="u", bufs=2) as up,
            tc.tile_pool(name="scr", bufs=2) as scrp,
            tc.tile_pool(name="small", bufs=1) as small,
            tc.tile_pool(name="psu", bufs=2, space="PSUM") as psu,
            tc.tile_pool(name="psr", bufs=1, space="PSUM") as psr,
        ):
            m_sb = consts.tile([128, CB * C], FP32, tag="m")
            nc.sync.dma_start(m_sb[:], m_in[:])
            ones1 = consts.tile([128, 1], FP32, tag="ones1")
            nc.vector.memset(ones1[:], 1.0)
            dotp = small.tile([128, n_dots], FP32, tag="dotp")

            def load_group(g):
                b = rl[g]
                a0 = starts[g]
                xg = xgp.tile([128, b * AG], FP32, tag="xg")
                src = x_in[a0:a0 + b].rearrange(
                    "b (cb p) s -> p b cb s", cb=CB, p=128)
                dst = xg[:].rearrange("p (b cb s) -> p b cb s",
                                      b=b, cb=CB, s=SP)
                nc.sync.dma_start(dst, src)
                return xg

            def compute_u(xg, j, utile):
                # utile[:, cb2*SP + s] = sum_c1 M[c1, cb2*128+p] * x[c1, s]
                for cb2 in range(CB):
                    for sc in range(n_spc):
                        pu = psu.tile([128, SPC], FP32, tag="pu")
                        for cb1 in range(CB):
                            nc.tensor.matmul(
                                pu[:],
                                m_sb[:, cb1 * C + cb2 * 128:
                                     cb1 * C + cb2 * 128 + 128],
                                xg[:, j * AG + cb1 * SP + sc * SPC:
                                   j * AG + cb1 * SP + sc * SPC + SPC],
                                start=(cb1 == 0), stop=(cb1 == CB - 1))
                        nc.scalar.copy(
                            utile[:, cb2 * SP + sc * SPC:
                                  cb2 * SP + sc * SPC + SPC], pu[:])

            def dot(xg, j, utile, col):
                scr = scrp.tile([128, AG], FP32, tag="scr")
                nc.vector.scalar_tensor_tensor(
                    out=scr[:], in0=xg[:, j * AG:(j + 1) * AG], scalar=SCALE,
                    in1=utile[:], op0=OP.mult, op1=OP.mult,
                    accum_out=dotp[:, col:col + 1])

            for g in range(n_groups):
                b = rl[g]
                xg = load_group(g)
                u0 = up.tile([128, AG], FP32, tag="u")
                compute_u(xg, 0, u0)
                for j in range(b):
                    dot(xg, j, u0, starts[g] + j)
                if g == n_groups - 1:
                    # extra rows for the last group's full att matrix
                    col = n_agents
                    for j1 in range(1, b):
                        uj = up.tile([128, AG], FP32, tag="u")
                        compute_u(xg, j1, uj)
                        for j2 in range(b):
                            dot(xg, j2, uj, col)
                            col += 1

            # partition-reduce all dot partials with one ones-matmul
            pr = psr.tile([1, n_dots], FP32, tag="pr")
            nc.tensor.matmul(pr[:], ones1[:], dotp[:], start=True, stop=True)
            dsb = small.tile([1, n_dots], FP32, tag="dsb")
            nc.scalar.copy(dsb[:], pr[:])
            nc.sync.dma_start(dots_out[:], dsb[:])

    nc.compile()
    return nc


def build_program_b(rl, C, SP):
    """Launch B: out_g^T = sum_j (att_j*Wvo)^T x_j + b_out."""
    n_agents = sum(rl)
    n_groups = len(rl)
    starts = np.concatenate([[0], np.cumsum(rl)]).astype(int)

    CB = C // 128
    AG = CB * SP
    SPC = 512
    n_spc = SP // SPC

    nc = bacc.Bacc("TRN2", target_bir_lowering=False, debug=False,
                   num_devices=N_CORES)
    x_in = nc.dram_tensor("xs", [n_agents, C, SP], FP32R, kind="ExternalInput")
    wvo_in = nc.dram_tensor("wvo", [128, CB * C], FP32, kind="ExternalInput")
    att_in = nc.dram_tensor("attbc", [128, n_agents], FP32, kind="ExternalInput")
    bout_in = nc.dram_tensor("bout", [1, C], FP32, kind="ExternalInput")
    out = nc.dram_tensor("out", [n_groups, C, SP], FP32, kind="ExternalOutput")

    with tile.TileContext(nc) as tc:
        with (
            tc.tile_pool(name="consts", bufs=1) as consts,
            tc.tile_pool(name="xg", bufs=3) as xgp,
            tc.tile_pool(name="wsc", bufs=12) as wscp,
            tc.tile_pool(name="osb", bufs=2) as osbp,
            tc.tile_pool(name="psz", bufs=2, space="PSUM") as psz,
        ):
            wvo_sb = consts.tile([128, CB * C], FP32, tag="wvo")
            nc.sync.dma_start(wvo_sb[:], wvo_in[:])
            att_sb = consts.tile([128, n_agents], FP32, tag="att")
            nc.sync.dma_start(att_sb[:], att_in[:])
            bout_sb = consts.tile([1, C], FP32, tag="bout")
            nc.sync.dma_start(bout_sb[:], bout_in[:])
            ones5 = consts.tile([1, SPC], FP32, tag="ones5")
            nc.vector.memset(ones5[:], 1.0)

            for g in range(n_groups):
                b = rl[g]
                a0 = starts[g]
                xg = xgp.tile([128, b * AG], FP32R, tag="xg")
                src = x_in[a0:a0 + b].rearrange(
                    "b (cb p) s -> p b cb s", cb=CB, p=128)
                dstv = xg[:].rearrange("p (b cb s) -> p b cb s",
                                       b=b, cb=CB, s=SP)
                nc.sync.dma_start(dstv, src)

                # att-scaled weights, rounded to f32r by the DVE write
                wst = []
                for j in range(b):
                    wsj = wscp.tile([128, CB * C], FP32R, tag="ws")
                    nc.vector.tensor_scalar(
                        out=wsj[:], in0=wvo_sb[:],
                        scalar1=att_sb[:, a0 + j:a0 + j + 1], scalar2=None,
                        op0=OP.mult)
                    wst.append(wsj)

                osb = osbp.tile([128, AG], FP32, tag="osb")
                for ob in range(CB):
                    for sc in range(n_spc):
                        pz = psz.tile([128, SPC], FP32, tag="pz")
                        # bias via K=1 matmul: b_out[ob] outer ones
                        nc.tensor.matmul(
                            pz[:], bout_sb[0:1, ob * 128:(ob + 1) * 128],
                            ones5[:], start=True, stop=False)
                        for j in range(b):
                            for cb in range(CB):
                                nc.tensor.matmul(
                                    pz[:],
                                    wst[j][:, cb * C + ob * 128:
                                           cb * C + ob * 128 + 128],
                                    xg[:, j * AG + cb * SP + sc * SPC:
                                       j * AG + cb * SP + sc * SPC + SPC],
                                    start=False,
                                    stop=(j == b - 1 and cb == CB - 1))
                        nc.scalar.copy(
                            osb[:, ob * SP + sc * SPC:
                                ob * SP + sc * SPC + SPC], pz[:])
                dst = out[g].rearrange("(cb p) s -> p cb s", cb=CB, p=128)
                osbv = osb[:].rearrange("p (cb s) -> p cb s", cb=CB, s=SP)
                nc.sync.dma_start(dst, osbv)

    nc.compile()
    return nc


_CACHE = {}


def _programs(rl_key, C, SP):
    if (rl_key, C, SP) not in _CACHE:
        rl = list(rl_key)
        _CACHE[(rl_key, C, SP)] = (build_program_a(rl, C, SP),
                                   build_program_b(rl, C, SP))
    return _CACHE[(rl_key, C, SP)]


def kernel(x, record_len, w_qkv, w_out, b_out, _timing=None):
    x = np.ascontiguousarray(np.asarray(x, dtype=np.float32))
    record_len = np.asarray(record_len)
    w_qkv = np.asarray(w_qkv, dtype=np.float32)
    w_out = np.asarray(w_out, dtype=np.float32)
    b_out = np.asarray(b_out, dtype=np.float32)

    N, C, H, W = x.shape
    SP = H * W // N_CORES
    rl, starts = _group_layout(record_len)
    n_agents = sum(rl)
    n_groups = len(rl)
    b_last = rl[-1]
    CB = C // 128

    wq = w_qkv[:, 0:C]
    wk = w_qkv[:, C:2 * C]
    wv = w_qkv[:, 2 * C:3 * C]
    M = (wq.astype(np.float64) @ wk.T.astype(np.float64)).astype(np.float32)
    Wvo = (wv.astype(np.float64) @ w_out.astype(np.float64)).astype(np.float32)

    # packed stationary layout: [:, cb*C + ob*128 : +128] = M[cb*128.., ob*128..]
    def pack(w):
        p = np.empty((128, CB * C), dtype=np.float32)
        for cb in range(CB):
            for ob in range(CB):
                p[:, cb * C + ob * 128: cb * C + ob * 128 + 128] = \
                    w[cb * 128:(cb + 1) * 128, ob * 128:(ob + 1) * 128]
        return p

    M_p = pack(M)
    Wvo_p = pack(Wvo)

    # per-core H slices, contiguous
    hs = H // N_CORES
    xs = [np.ascontiguousarray(
        x[:, :, c * hs:(c + 1) * hs, :]).reshape(N, C, SP)
        for c in range(N_CORES)]

    prog_a, prog_b = _programs(tuple(rl), C, SP)

    import time
    t0 = time.monotonic()
    in_maps_a = [{"xs": xs[c], "m": M_p} for c in range(N_CORES)]
    res_a = run_bass_kernel_spmd(prog_a, in_maps_a, list(range(N_CORES)))
    t1 = time.monotonic()

    n_extra = (b_last - 1) * b_last
    n_dots = n_agents + n_extra
    part = np.zeros((1, n_dots), dtype=np.float64)
    for c in range(N_CORES):
        part += res_a.results[c]["dots"].astype(np.float64)
    logits_row0 = part[0, :n_agents]

    # softmax per group (float64, exact)
    att = np.zeros(n_agents, dtype=np.float32)
    for g in range(n_groups):
        lg = logits_row0[starts[g]:starts[g + 1]]
        e = np.exp(lg - lg.max())
        att[starts[g]:starts[g + 1]] = (e / e.sum()).astype(np.float32)

    # full att matrix of the last group
    bl = b_last
    att_last = np.zeros((bl, bl), dtype=np.float32)
    lrow = logits_row0[starts[-2]:starts[-1]]
    e = np.exp(lrow - lrow.max())
    att_last[0] = (e / e.sum()).astype(np.float32)
    extra = part[0, n_agents:].reshape(bl - 1, bl)
    for j1 in range(1, bl):
        lg = extra[j1 - 1]
        e = np.exp(lg - lg.max())
        att_last[j1] = (e / e.sum()).astype(np.float32)

    att_bc = np.repeat(att[None, :], 128, axis=0).astype(np.float32)

    t2 = time.monotonic()
    in_maps_b = [{"xs": xs[c], "wvo": Wvo_p, "attbc": att_bc,
                  "bout": b_out.reshape(1, C)} for c in range(N_CORES)]
    res_b = run_bass_kernel_spmd(prog_b, in_maps_b, list(range(N_CORES)))
    t3 = time.monotonic()

    out = np.empty((n_groups, C, H, W), dtype=np.float32)
    for c in range(N_CORES):
        out[:, :, c * hs:(c + 1) * hs, :] = \
            res_b.results[c]["out"].reshape(n_groups, C, hs, W)

    if _timing is not None:
        _timing["wall_a_s"] = t1 - t0
        _timing["wall_b_s"] = t3 - t2
        _timing["exec_a_ns"] = res_a.exec_time_ns
        _timing["exec_b_ns"] = res_b.exec_time_ns

    return out, att_last
